# revision 44
# baseline (speedup 1.0000x reference)
"""Trainium2 Bass kernel for the SelfOrg spiking-network step.

Reference computation (per batch b, neuron n):
    z_out_new = BETA * z_out + z
    z_loo[b,j,n] = z_out_new[b, j + (j>=n)]            (leave-one-out gather)
    drive[b,n]  = sum_k x[b,k,n] * w[k,n]  (k < N_IN)
                + sum_j z_loo[b,j,n] * w[N_IN+j, n]
    v_new = ALPHA*v + drive - V_TH*z
    z_new = (v_new - V_TH > 0)

Strategy:
  * Batch-parallel over 8 cores (8 batches each). The kernel is memory
    bound on streaming x; x and w are cast to fp16 on the host, which
    halves HBM traffic (error ~2^-11 per term, far inside the 2e-2
    tolerance) and unlocks the DVE 2x packed mode and the PE 16-bit
    1-column/cycle rate.
  * Pipeline per chunk: DMA -> DVE product (tm = x*w, folding the last
    slice pair on alternate chunks) -> PE reduces the 512-column slices
    over partitions with a per-batch indicator stationary operand
    (lhsT[p, m] = (m==b)) into one (8,512) PSUM tile. The partial fold
    balances DVE (~229 G elem/s fp16) against PE (~380ns per matmul
    incl. LDWEIGHTS), keeping both under the ~52us DMA roofline.
  * The leave-one-out term is algebraically a dense matmul
    z_out_new @ Wf where Wf[m,n] = w[N_IN + m - (m>n), n], diag(Wf)=0.
    Wf is precomputed on the host; its 4 matmuls run mid-stream into
    their own PSUM tile, off the critical path.
  * DMA chunks alternate strictly between the two HWDGE rings
    (sync + scalar/ACT): each sequencer then carries half the
    descriptor-issue (~0.6us) and buffer-recycle semaphore load, and
    because consecutive chunks are needed back-to-back, the rings'
    fair-share of the 16 SDMA engines matches the consumption order.
"""

import numpy as np

# model hyperparameters (must match the reference)
N_IN = 2048
NN = 512
BATCH = 64
DT, TAU_M, TAU_X = 0.05, 10.0, 2.0
ALPHA = 1.0 - DT / TAU_M
BETA = 1.0 - DT / TAU_X
V_TH = 2.0

NCORES = 8
BPC = BATCH // NCORES      # batches per core
P = 128                    # SBUF partitions
S = N_IN // P              # 16 k-rows folded per partition
FD = S * NN                # 8192 free elements of one batch tile
CHUNKS = 4                 # chunks per batch
CFD = FD // CHUNKS         # 2048 free elements per chunk
SPC = S // CHUNKS          # 4 slices per chunk (last pair folded -> 3 mms)


def _build_nc():
    import concourse.mybir as mybir
    from concourse import bacc
    from concourse.masks import make_identity
    from concourse.tile import TileContext

    f32 = mybir.dt.float32
    f16 = mybir.dt.float16
    nc = bacc.Bacc("TRN2", name="selforg_step")

    x_h = nc.dram_tensor("x", [BPC, N_IN, NN], f16, kind="ExternalInput")
    st_h = nc.dram_tensor("state", [3, BPC, NN], f32, kind="ExternalInput")
    w_h = nc.dram_tensor("w", [N_IN, NN], f16, kind="ExternalInput")
    wf_h = nc.dram_tensor("wf", [NN, NN], f16, kind="ExternalInput")
    out_h = nc.dram_tensor("out", [BPC, 3, NN], f32, kind="ExternalOutput")

    # partition p <- x[b] bytes [16KB*p, 16KB*(p+1)): k = 16p + s
    x_r = x_h[:, :, :].rearrange("b (p s) n -> b p (s n)", p=P)
    w_r = w_h[:, :].rearrange("(p s) n -> p (s n)", p=P)
    wf_r = wf_h[:, :].rearrange("(t p) n -> p t n", p=P)
    st_r = st_h[:, :, :].rearrange("t b n -> b t n")
    out_r = out_h[:, :, :].rearrange("b t n -> b (t n)")

    with TileContext(nc) as tc:
        with (
            tc.tile_pool(name="const", bufs=1) as cpool,
            tc.tile_pool(name="xin", bufs=10) as xpool,
            tc.tile_pool(name="tmp", bufs=8) as tpool,
            tc.tile_pool(name="psum", bufs=1, space="PSUM") as ppool,
            tc.tile_pool(name="psum2", bufs=2, space="PSUM") as ppool2,
            tc.tile_pool(name="psum3", bufs=1, space="PSUM") as ppool3,
        ):
            # ---- input DMAs: state first (sync ring), then w/x chunks
            # alternating between the two HWDGE rings in need-order.
            st_sb = cpool.tile([BPC, 3 * NN], f32)
            zo_in = st_sb[:, 0:NN]
            z_in = st_sb[:, NN : 2 * NN]
            v_in = st_sb[:, 2 * NN : 3 * NN]
            nc.sync.dma_start(st_sb[:, :].rearrange("b (t n) -> b t n", t=3), st_r)
            w_sb = cpool.tile([P, FD], f16)
            wf_sb = cpool.tile([P, 4 * NN], f16)

            ind = cpool.tile([P, BPC * BPC], f16)
            nc.gpsimd.memset(ind[:, :], 0.0)
            for b in range(BPC):
                nc.gpsimd.memset(ind[:, (BPC + 1) * b : (BPC + 1) * b + 1], 1.0)

            ident = cpool.tile([BPC, BPC], f32)
            make_identity(nc, ident[:, :])

            # ---- output staging tile: [vn | zn | zon] in the free dim
            res = cpool.tile([BPC, 3 * NN], f32)
            vn = res[:, 0:NN]
            zn = res[:, NN : 2 * NN]
            zon = res[:, 2 * NN : 3 * NN]

            zonT = cpool.tile([P, 4 * BPC], f16)
            av_sb = cpool.tile([BPC, NN], f32)
            zv_sb = cpool.tile([BPC, NN], f32)

            # ---- main loop: drive[b,n] = sum_k x[b,k,n]*w[k,n] ----
            # Batch 0 streams in 8 quarter-chunks (fast ramp), middle
            # batches in 1MB halves (fewest issues/semaphores), the last
            # batch in 512KB chunks (short drain). Alternate chunks fold
            # their last slice pair on the DVE so DVE and PE stay
            # balanced under the DMA roofline.
            psum_drive = ppool.tile([BPC, NN], f32, tag="drive")
            lat_tile = ppool3.tile([BPC, NN], f32, tag="lat")
            def _chunks_for(b):
                # fast ramp, fat middle (fewer issues/semaphores), drain
                if b == 0:
                    return 2 * CHUNKS
                if b == BPC - 1:
                    return CHUNKS
                return CHUNKS // 2

            total_mms = 0
            _ci = 0
            for _b in range(BPC):
                _chunks = _chunks_for(_b)
                _spc = FD // _chunks // NN
                for _c in range(_chunks):
                    total_mms += (_spc - 1) if (_b > 0 and _ci % 2 == 0) else _spc
                    _ci += 1
            mm_idx = 0
            ci = 0
            for b in range(BPC):
                chunks = _chunks_for(b)
                cfd = FD // chunks
                for c in range(chunks):
                    eng = nc.sync if ci % 2 == 0 else nc.scalar
                    cs = slice(c * cfd, (c + 1) * cfd)
                    if b == 0:
                        # w chunk rides the opposite ring, in parallel
                        weng = nc.scalar if ci % 2 == 0 else nc.sync
                        weng.dma_start(w_sb[:, cs], w_r[:, cs])
                    xc = xpool.tile([P, cfd], f16, tag="xc")
                    eng.dma_start(xc[:, :], x_r[b, :, cs])
                    if b == 0 and c == chunks - 1:
                        # wf rides mid-stream; only the lateral mms need it
                        nc.scalar.dma_start(
                            wf_sb[:, :].rearrange("p (t n) -> p t n", t=4),
                            wf_r[:, :, :],
                        )
                    tm = tpool.tile([P, cfd], f16, tag="tm")
                    nc.vector.tensor_mul(tm[:, :], xc[:, :], w_sb[:, cs])
                    # fold the last slice pair on alternate chunks only,
                    # balancing DVE and PE work under the DMA roofline
                    spc = cfd // NN
                    fold = b > 0 and ci % 2 == 0
                    if fold:
                        nc.vector.tensor_add(
                            tm[:, (spc - 2) * NN : (spc - 1) * NN],
                            tm[:, (spc - 2) * NN : (spc - 1) * NN],
                            tm[:, (spc - 1) * NN : spc * NN],
                        )
                    for j in range(spc - 1 if fold else spc):
                        nc.tensor.matmul(
                            psum_drive[:, :],
                            ind[:, BPC * b : BPC * (b + 1)],
                            tm[:, j * NN : (j + 1) * NN],
                            start=(mm_idx == 0),
                            stop=(mm_idx == total_mms - 1),
                        )
                        mm_idx += 1
                    ci += 1
                if b == 1:
                    # mid-stream slack: trace update + PE transposes of
                    # zon (4x (8,128) -> (128,8), cast fp16) + av
                    nc.vector.tensor_scalar_mul(zon, zo_in, BETA)
                    nc.vector.tensor_add(zon, zon, z_in)
                    for t in range(4):
                        psum_t = ppool2.tile([P, BPC], f32, tag="tr")
                        nc.tensor.transpose(
                            psum_t[:, :], zon[:, t * P : (t + 1) * P], ident[:, :]
                        )
                        nc.vector.tensor_copy(
                            zonT[:, t * BPC : (t + 1) * BPC], psum_t[:, :]
                        )
                    nc.vector.tensor_scalar_mul(av_sb[:, :], z_in, -V_TH)
                    nc.vector.tensor_scalar_mul(zv_sb[:, :], v_in, ALPHA)
                    nc.vector.tensor_add(av_sb[:, :], av_sb[:, :], zv_sb[:, :])
                if b == 2:
                    # lateral drive, off the critical path
                    for t in range(4):
                        nc.tensor.matmul(
                            lat_tile[:, :],
                            zonT[:, t * BPC : (t + 1) * BPC],
                            wf_sb[:, t * NN : (t + 1) * NN],
                            start=(t == 0),
                            stop=(t == 3),
                        )

            # ---- epilogue ----
            nc.vector.tensor_add(vn, av_sb[:, :], lat_tile[:, :])
            nc.vector.tensor_add(vn, vn, psum_drive[:, :])
            nc.vector.tensor_scalar(
                out=zn,
                in0=vn,
                scalar1=V_TH,
                scalar2=None,
                op0=mybir.AluOpType.is_gt,
            )
            nc.scalar.dma_start(out_r, res[:, :])

    return nc


def _make_wf(w: np.ndarray) -> np.ndarray:
    """Wf[m,n] = w[N_IN + m - (m>n), n] off-diagonal, 0 on the diagonal."""
    wl = w[N_IN:]
    m = np.arange(NN)[:, None]
    n = np.arange(NN)[None, :]
    idx = np.minimum(np.where(m > n, m - 1, m), NN - 2)
    return np.where(m == n, np.float32(0.0), wl[idx, n]).astype(np.float32)


def _make_in_maps(x, v, z, z_out, w):
    w16 = np.ascontiguousarray(w[:N_IN]).astype(np.float16)
    wf16 = _make_wf(np.asarray(w, dtype=np.float32)).astype(np.float16)
    x16 = np.asarray(x).astype(np.float16)
    state = np.stack(
        [
            np.asarray(z_out, dtype=np.float32),
            np.asarray(z, dtype=np.float32),
            np.asarray(v, dtype=np.float32),
        ]
    )
    in_maps = []
    for c in range(NCORES):
        sl = slice(c * BPC, (c + 1) * BPC)
        in_maps.append(
            {
                "x": np.ascontiguousarray(x16[sl]),
                "state": np.ascontiguousarray(state[:, sl]),
                "w": w16,
                "wf": wf16,
            }
        )
    return in_maps


def run(x, v, z, z_out, w, trace=False):
    """Build + run on the 8 NeuronCores; returns (output, BassKernelResults)."""
    from concourse.bass_utils import run_bass_kernel_spmd

    nc = _build_nc()
    if not nc.is_finalized():
        nc.finalize()
    in_maps = _make_in_maps(x, v, z, z_out, w)
    res = run_bass_kernel_spmd(nc, in_maps, core_ids=list(range(NCORES)), trace=trace)
    # per-core out is [BPC, 3, NN]; reassemble to [3, BATCH, NN]
    full = np.concatenate([r["out"].transpose(1, 0, 2) for r in res.results], axis=1)
    return np.ascontiguousarray(full, dtype=np.float32), res


def kernel(x, v, z, z_out, w):
    out, _ = run(x, v, z, z_out, w)
    return out
